# revision 3
# baseline (speedup 1.0000x reference)
"""Trainium2 Bass kernel for CNCAgg (weight-MLP + bmm aggregation + final 1x1 conv).

v3 strategy (8 cores, data-parallel over B=32, NO collectives):
  - Per core: 4 batches. WeightNet MLP in bf16 with 2x2 PE-quadrant packing
    (4 batches concurrent). L3 emits transposed (n on partitions) for the bmm.
  - feature is pre-transposed/quantized on host to fp8 e4m3 (4.2 MB/core).
  - bmm = fp8 feat x bf16 wgt, accumulated fp32; agg shuffled to cw-major
    bf16 (agg_s[p, kc, b]).
  - Final conv weight wf' (512 x 16384, BN scale + 1/N folded in) is
    REPLICATED per core in fp8 e4m3 (8.4 MB/core), streamed through SBUF;
    the conv is a single 128-step PSUM accumulation chain
    psO[4, 512] += agg_s[:, kc, :]^T @ wf[kc].  The fp8 scale 2^k on wf is
    compensated by scaling the L3 weights by 2^-k (ReLU is positively
    homogeneous).
  - No cross-core communication: each core writes its own (4, 512) output;
    host concatenates.  Max core time is independent of SPMD launch stagger.
"""

import os
import sys

sys.path.insert(0, "/opt/trn_rl_repo")

KSTAGE = int(os.environ.get("KSTAGE", "4"))

import numpy as np
import ml_dtypes

import concourse.bass as bass
from concourse import bacc
import concourse.mybir as mybir
from concourse.bass import ds, ts
from concourse.tile import TileContext
from concourse.bass_utils import run_bass_kernel_spmd

# ---------------------------------------------------------------- constants
B, N, C, OUT, W = 32, 4096, 256, 512, 64
EPS = 1e-5
NCORES = 8
BLOC = B // NCORES            # 4 batches per core
KCW = C * W                   # 16384 contraction dim of final conv
NKC = KCW // 128              # 128 cw-chunks of 128
NCH = N // 128                # 32 n-chunks of 128 per batch
FTI = N // 1024               # 4 feature tiles per batch (1024 pts each)
WFG = 4                       # kc-chunks per wf DMA tile
NWFT = NKC // WFG             # 32 wf DMA tiles

F32 = mybir.dt.float32
BF16 = mybir.dt.bfloat16
F8 = mybir.dt.float8e4
NPBF = ml_dtypes.bfloat16
NPF8 = ml_dtypes.float8_e4m3
RELU = mybir.ActivationFunctionType.Relu
ALU = mybir.AluOpType


def build_bass():
    nc = bacc.Bacc("TRN2", target_bir_lowering=False, debug=True,
                   num_devices=NCORES)

    # per-core inputs
    x3_d = nc.dram_tensor("x3p", [BLOC, 3, N], BF16, kind="ExternalInput")
    # featT[b, i, p, jj, c] = feature^T[b, n=1024*i+128*jj+p, c] (fp8)
    ft_d = nc.dram_tensor("featT", [BLOC, FTI, 128, 8, C], F8,
                          kind="ExternalInput")
    w1_d = nc.dram_tensor("w1t", [128, W], BF16, kind="ExternalInput")
    w2_d = nc.dram_tensor("w2t", [128, W], BF16, kind="ExternalInput")
    w3_d = nc.dram_tensor("w3t", [128, W], BF16, kind="ExternalInput")
    b1_d = nc.dram_tensor("b1", [128, 1], F32, kind="ExternalInput")
    b2_d = nc.dram_tensor("b2", [128, 1], F32, kind="ExternalInput")
    b3_d = nc.dram_tensor("b3rep", [128, 8, W], F32, kind="ExternalInput")
    # wfT[t, p, j, o] = wf'^T[cw_lin = 128*(4t+j) + p, o] (fp8, full)
    wf_d = nc.dram_tensor("wfT", [NWFT, 128, WFG, OUT], F8,
                          kind="ExternalInput")
    bf_d = nc.dram_tensor("bfrep", [BLOC, OUT], F32, kind="ExternalInput")
    out_d = nc.dram_tensor("out", [BLOC, OUT], F32, kind="ExternalOutput")
    agdbg_d = nc.dram_tensor("aggdbg", [128, NKC, BLOC], F32,
                             kind="ExternalOutput") if KSTAGE >= 90 else None

    with TileContext(nc) as tc:
        with (
            tc.tile_pool(name="const", bufs=1) as cpool,
            tc.tile_pool(name="hbuf", bufs=1) as hpool,
            tc.tile_pool(name="wgt", bufs=2) as wpool,
            tc.tile_pool(name="feat", bufs=16) as fpool,
            tc.tile_pool(name="wfin", bufs=8) as wfpool,
            tc.tile_pool(name="osb", bufs=1) as opool,
            tc.tile_pool(name="ph", bufs=2, space="PSUM") as pph,
            tc.tile_pool(name="pw", bufs=2, space="PSUM") as ppw,
            tc.tile_pool(name="pa", bufs=1, space="PSUM") as ppa,
            tc.tile_pool(name="pf", bufs=1, space="PSUM") as ppf,
        ):
            # ---- constants; w1t + x3 first (L1-critical)
            w1t = cpool.tile([128, W], BF16, tag="w1t")
            nc.sync.dma_start(out=w1t[:], in_=w1_d[:])
            x3 = cpool.tile([128, N], BF16, tag="x3")
            for b in range(BLOC):
                nc.sync.dma_start(out=x3[ds(32 * b, 3), :], in_=x3_d[b])
            w2t = cpool.tile([128, W], BF16, tag="w2t")
            nc.scalar.dma_start(out=w2t[:], in_=w2_d[:])
            w3t = cpool.tile([128, W], BF16, tag="w3t")
            nc.scalar.dma_start(out=w3t[:], in_=w3_d[:])
            b1t = cpool.tile([128, 1], F32, tag="b1")
            nc.scalar.dma_start(out=b1t[:], in_=b1_d[:])
            b2t = cpool.tile([128, 1], F32, tag="b2")
            nc.scalar.dma_start(out=b2t[:], in_=b2_d[:])
            b3t = cpool.tile([128, 8, W], F32, tag="b3")
            nc.scalar.dma_start(out=b3t[:], in_=b3_d[:])
            bft = cpool.tile([BLOC, OUT], F32, tag="bf")
            nc.scalar.dma_start(out=bft[:], in_=bf_d[:])
            # agg_s[p, kc, b]: agg[cw_lin = 128*kc + p, batch b] (bf16)
            agg_s = cpool.tile([128, NKC, BLOC], BF16, tag="aggs")
            zeros = cpool.tile([128, 2, 256], BF16, tag="zeros")
            nc.vector.memset(zeros[:], 0.0)

            # ---- feature tiles: (128, 8, C) fp8, FTI per batch, streamed
            ft_tiles = {}

            def load_ft(b, i):
                ft = fpool.tile([128, 8, C], F8, tag="ft", name=f"ft{b}_{i}")
                nc.sync.dma_start(out=ft[:], in_=ft_d[b, i])
                ft_tiles[(b, i)] = ft

            # stream order: pair (0,1) interleaved first
            for i in range(FTI):
                load_ft(0, i)
                load_ft(1, i)

            # ---- wf tiles (replicated fp8 conv weight), on scalar queue
            wf_tiles = []

            def load_wf(t):
                wt = wfpool.tile([128, WFG, OUT], F8, tag="wf", name=f"wf{t}")
                nc.scalar.dma_start(out=wt[:], in_=wf_d[t])
                wf_tiles.append(wt)

            for t in range(NWFT):
                load_wf(t)

            # ---- L1: 3 -> 64, 4 batches on PE quadrants
            # x3 strips: batch b on partitions 32b..32b+2
            # quadrants: b0 (0,0)->ps[0:64, 0]; b1 (32,0)->ps[0:64, 1];
            #            b2 (64,64)->ps[64:128, 0]; b3 (96,64)->ps[64:128, 1]
            # h layout h[p, half, n]: half0 = {b0 low, b2 high}, half1 = {b1, b3}
            h1 = hpool.tile([128, 2, N], BF16, tag="h1")
            h2 = hpool.tile([128, 2, N], BF16, tag="h2")
            # concurrent row-tiled matmuls MUST land in different PSUM banks:
            # even batches -> bank A, odd batches -> bank B (each a full bank)
            FCH = 512
            for i in range(N // FCH):
                psA = pph.tile([128, FCH], F32, tag="hpsA")
                psB = pph.tile([128, FCH], F32, tag="hpsB")
                for b in range(BLOC):
                    row = 32 * b
                    col = 64 * (b // 2)
                    ps = psA if b % 2 == 0 else psB
                    nc.tensor.matmul(
                        ps[ds(col, W), :],
                        lhsT=w1t[ds(row, 3), :],
                        rhs=x3[ds(row, 3), ds(i * FCH, FCH)],
                        start=True, stop=True,
                        tile_position=(row, col), skip_group_check=True,
                    )
                nc.scalar.activation(
                    h1[:, 0, ds(i * FCH, FCH)], psA[:], RELU, bias=b1t[:]
                )
                nc.vector.scalar_tensor_tensor(
                    h1[:, 1, ds(i * FCH, FCH)],
                    in0=psB[:], scalar=b1t[:], in1=zeros[:],
                    op0=ALU.add, op1=ALU.max,
                )

            # ---- L2: 64 -> 64, 4 batches on quadrants
            # L1 emits batch b at h1[64*(b//2) rows, free-half b%2].
            # L2 quadrants (srow = h1 strip, col = out partitions):
            #   b0 (0,0)->A[0:64], b1 (0,64)->A[64:128],
            #   b2 (64,0)->B[0:64], b3 (64,64)->B[64:128]
            # => h2[:,0] = [b0|b1], h2[:,1] = [b2|b3]:
            #    batch b at h2[64*(b%2) rows, free-half b//2] (what L3 wants)
            for i in range(N // FCH):
                psA = pph.tile([128, FCH], F32, tag="hpsA")
                psB = pph.tile([128, FCH], F32, tag="hpsB")
                for b in range(BLOC):
                    srow = 64 * (b // 2)     # h1 partition strip of batch b
                    col = 64 * (b % 2)
                    ps = psA if b < 2 else psB
                    nc.tensor.matmul(
                        ps[ds(col, W), :],
                        lhsT=w2t[ds(srow, W), :],
                        rhs=h1[ds(srow, W), b % 2, ds(i * FCH, FCH)],
                        start=True, stop=True,
                        tile_position=(srow, col), skip_group_check=True,
                    )
                nc.vector.scalar_tensor_tensor(
                    h2[:, 0, ds(i * FCH, FCH)],
                    in0=psA[:], scalar=b2t[:], in1=zeros[:],
                    op0=ALU.add, op1=ALU.max,
                )
                nc.scalar.activation(
                    h2[:, 1, ds(i * FCH, FCH)], psB[:], RELU, bias=b2t[:]
                )

            # ---- L3 (transposed out): per batch, wgt[b] = (n x w) bf16
            # batch b: h2 strip rows 64*(b%2), free-half b//2
            wgt_tiles = [None] * BLOC

            def l3_group(b, j):
                """8 n-chunks (j*8 .. j*8+8) of batch b's transposed L3."""
                row = 64 * (b % 2)
                if wgt_tiles[b] is None:
                    wgt_tiles[b] = wpool.tile([128, NCH, W], BF16, tag="wgt",
                                              name=f"wgt{b}")
                wgt = wgt_tiles[b]
                pwg = ppw.tile([128, 8, W], F32, tag="wps")
                for jj in range(8):
                    i = j * 8 + jj
                    nc.tensor.matmul(
                        pwg[:, jj, :],
                        lhsT=h2[ds(row, W), b // 2, ds(i * 128, 128)],
                        rhs=w3t[ds(row, W), :],
                        start=True, stop=True,
                        tile_position=(row, 0), skip_group_check=True,
                    )
                nc.vector.tensor_add(pwg[:], pwg[:], b3t[:])
                nc.scalar.activation(wgt[:, ts(j, 8), :], pwg[:], RELU)

            # ---- bmm for a batch pair (be, bo) = (2q, 2q+1), col-tiled 2x
            # b even -> col 0 (pa[0:64]), b odd -> col 64 (pa[64:128])
            def bmm_pair(q, interleave=None):
                be, bo = 2 * q, 2 * q + 1
                pa = ppa.tile([128, C], F32, tag="aps")
                for i in range(FTI):
                    for b, col in ((be, 0), (bo, 64)):
                        if (b, i) not in ft_tiles:
                            load_ft(b, i)
                        ftile = ft_tiles[(b, i)]
                        wgt = wgt_tiles[b]
                        for jj in range(8):
                            ch = 8 * i + jj
                            nc.tensor.matmul(
                                pa[ds(col, W), :],
                                lhsT=wgt[:, ch, :],
                                rhs=ftile[:, jj, :],
                                start=(ch == 0), stop=(ch == NCH - 1),
                                tile_position=(0, col),
                                skip_group_check=True,
                            )
                    if interleave is not None:
                        interleave(i)
                # shuffle into agg_s: agg_s[64*(c%2)+w, c//2, b] = pa[wrow, c]
                for b, base in ((be, 0), (bo, 64)):
                    pav = pa[ds(base, W), :].rearrange("w (k two) -> w two k",
                                                       two=2)
                    nc.vector.tensor_copy(agg_s[0:W, :, b], pav[:, 0, :])
                    nc.vector.tensor_copy(agg_s[W:128, :, b], pav[:, 1, :])

            # L3 for pair 0 up front; pair 1's L3 interleaves into bmm(0)
            for j in range(4):
                l3_group(0, j)
            for j in range(4):
                l3_group(1, j)

            def inter0(step):
                # compute pair-1 L3 inside bmm(0): 2 groups per bmm step
                b = 2 if step < 2 else 3
                l3_group(b, 2 * (step % 2))
                l3_group(b, 2 * (step % 2) + 1)

            # stream pair-1 features behind pair 0
            for i in range(FTI):
                load_ft(2, i)
                load_ft(3, i)

            bmm_pair(0, interleave=inter0)
            bmm_pair(1)

            if KSTAGE >= 90:
                agf = opool.tile([128, NKC, BLOC], F32, tag="agf")
                nc.vector.tensor_copy(agf[:], agg_s[:])
                nc.sync.dma_start(out=agdbg_d[:], in_=agf[:])

            # ---- final conv: psO[4, 512] += agg_s[:, kc, :].T @ wf[kc]
            psO = ppf.tile([BLOC, OUT], F32, tag="fps")
            for t in range(NWFT):
                wt = wf_tiles[t]
                for j in range(WFG):
                    kc = WFG * t + j
                    nc.tensor.matmul(
                        psO[:],
                        lhsT=agg_s[:, kc, :],
                        rhs=wt[:, j, :],
                        start=(kc == 0), stop=(kc == NKC - 1),
                    )
            Fo = opool.tile([BLOC, OUT], F32, tag="Fo")
            nc.vector.tensor_add(Fo[:], psO[:], bft[:])
            G = opool.tile([BLOC, OUT], F32, tag="G")
            nc.scalar.activation(G[:], Fo[:], RELU)
            nc.sync.dma_start(out=out_d[:], in_=G[:])

    nc.compile()
    return nc


_NC_CACHE = None


def _get_nc():
    global _NC_CACHE
    if _NC_CACHE is None:
        _NC_CACHE = build_bass()
    return _NC_CACHE


def _fold_bn(w, b, g, be, m, v):
    """Fold eval-mode BN into conv weight/bias: y = diag(s) W x + (s*(b-m)+be)."""
    s = (g / np.sqrt(v + EPS)).astype(np.float64)
    wp = (w.astype(np.float64) * s[:, None]).astype(np.float32)
    bp = (s * (b.astype(np.float64) - m) + be).astype(np.float32)
    return wp, bp


def prep_inputs(xyz, feature, w1, b1, g1, be1, m1, v1,
                w2, b2, g2, be2, m2, v2,
                w3, b3, g3, be3, m3, v3,
                wf, bf, gf, bef, mf, vf):
    """Host-side prep: BN folding, transposes, fp8 quantization, sharding."""
    w1p, b1p = _fold_bn(w1, b1, g1, be1, m1, v1)
    w2p, b2p = _fold_bn(w2, b2, g2, be2, m2, v2)
    w3p, b3p = _fold_bn(w3, b3, g3, be3, m3, v3)
    wfp, bfp = _fold_bn(wf, bf, gf, bef, mf, vf)
    # 1/N feature scaling folded into the final conv weight
    wfp = (wfp / N).astype(np.float32)
    # fp8 scale for wf: 2^k, compensated by scaling the L3 output by 2^-k
    k = int(np.floor(np.log2(448.0 / np.abs(wfp).max()))) - 2
    S = np.float32(2.0 ** k)
    wfq = (wfp * S).astype(np.float32)
    w3p = (w3p / S).astype(np.float32)
    b3p = (b3p / S).astype(np.float32)

    # w1t strips: w1p.T at partition rows {0,32,64,96}
    w1t = np.zeros((128, W), dtype=np.float32)
    for b in range(BLOC):
        w1t[32 * b:32 * b + 3] = w1p.T
    # wfT permuted rows: cw_lin(c, w) = 128*(c//2) + 64*(c%2) + w
    cw = np.arange(KCW)
    c_idx = cw // W
    w_idx = cw % W
    cw_lin = 128 * (c_idx // 2) + 64 * (c_idx % 2) + w_idx
    wfT_perm = np.empty((KCW, OUT), dtype=np.float32)
    wfT_perm[cw_lin] = wfq.T          # row cw_lin <- wfq[:, c*64+w]
    # [t, p, j, o] = wfT_perm[128*(4t+j) + p, o]
    wfT = wfT_perm.reshape(NWFT, WFG, 128, OUT).transpose(0, 2, 1, 3)

    shared = {
        "w1t": w1t.astype(NPBF),
        "w2t": np.ascontiguousarray(np.tile(w2p.T, (2, 1))).astype(NPBF),
        "w3t": np.ascontiguousarray(np.tile(w3p.T, (2, 1))).astype(NPBF),
        "b1": np.tile(b1p, 2).reshape(128, 1).astype(np.float32),
        "b2": np.tile(b2p, 2).reshape(128, 1).astype(np.float32),
        "b3rep": np.tile(b3p, (128, 8, 1)).astype(np.float32),
        "bfrep": np.tile(bfp, (BLOC, 1)).astype(np.float32),
        "wfT": np.ascontiguousarray(wfT).astype(NPF8),
    }
    in_maps = []
    for core in range(NCORES):
        xs = xyz[core * BLOC:(core + 1) * BLOC]        # (4, 4096, 3)
        x3p = np.ascontiguousarray(xs.transpose(0, 2, 1))  # (4, 3, N)
        fs = feature[core * BLOC:(core + 1) * BLOC]    # (4, 256, 4096)
        ftT = fs.transpose(0, 2, 1)                    # (4, 4096, 256)
        # [b, i, p, jj, c] = featT[b, 1024*i + 128*jj + p, c]
        ftT = ftT.reshape(BLOC, FTI, 8, 128, C).transpose(0, 1, 3, 2, 4)
        in_maps.append({
            "x3p": x3p.astype(NPBF),
            "featT": np.ascontiguousarray(ftT).astype(NPF8),
            **shared,
        })
    return in_maps


def _run(inputs, trace=False):
    inputs = {k: np.asarray(v) for k, v in inputs.items()}
    nc = _get_nc()
    in_maps = prep_inputs(
        inputs["xyz"], inputs["feature"],
        inputs["w1"], inputs["b1"], inputs["g1"], inputs["be1"], inputs["m1"], inputs["v1"],
        inputs["w2"], inputs["b2"], inputs["g2"], inputs["be2"], inputs["m2"], inputs["v2"],
        inputs["w3"], inputs["b3"], inputs["g3"], inputs["be3"], inputs["m3"], inputs["v3"],
        inputs["wf"], inputs["bf"], inputs["gf"], inputs["bef"], inputs["mf"], inputs["vf"],
    )
    res = run_bass_kernel_spmd(
        nc, in_maps, core_ids=list(range(NCORES)), trace=trace,
        trace_cores=list(range(NCORES)) if trace else None,
    )
    outs = [np.asarray(res.results[i]["out"]).reshape(BLOC, OUT)
            for i in range(NCORES)]
    full = np.concatenate(outs, axis=0).astype(np.float32)             # (32, 512)
    return full.reshape(B, OUT, 1), res


def kernel(**inputs):
    return _run(inputs, trace=False)[0]
